# revision 38
# baseline (speedup 1.0000x reference)
"""Sparse (Cantor-coordinate k-NN) attention on 8 Trainium2 NeuronCores.

Strategy (v6: sequence-parallel, zero-collective)
-------------------------------------------------
The k-NN routing in 1-D Cantor-coordinate space selects, for each query, a
contiguous window of 128 keys in *sorted-coordinate order*.  Tokens are
permuted into sorted order on the host (pure data movement) and the device
runs *banded* dense attention: each 128-query block attends to a 128-aligned
key slab of width R, with an additive -1e30 mask encoding each query's exact
128-key window.

Sharding: core c handles batch c//4 and sorted-token quarter c%4 (512
queries, ALL 8 heads).  Keys/values are computed only over the core's
contiguous key halo (KH tokens, host-sliced and zero-padded).  Because all
heads are local, the out-projection contracts the full 512 dims locally --
no cross-core reduction and no collectives.  The host merely slices inputs
per core and concatenates + inverse-permutes the outputs.

Device kernel per core (fp32r matmuls, bf16 probs/V):
  xq/xk -> PE-transpose -> x^T ; QKV projection (q^T,k^T head-transposed,
  v natural with interleaved ones columns for fused rowsum) ;
  per (block, head) unit, software-pipelined: scores (fp32r), mask-add
  (DVE), exp (ACT, bf16), probs transpose (PE, packed psum evictions),
  AV+rowsum (bf16 PE), normalize on eviction, out^T assembly ;
  out-proj (fp32r, bias folded via rank-1 matmul).
"""

import numpy as np

S = 2048
D = 512
HEADS = 8
HD = 64
KNN = 128
QB = 128              # queries per block
NBLK = S // QB
QPC = 512             # queries per core
BPC = QPC // QB       # blocks per core = 4
SCALE = 1.0 / (HD ** 0.5)
NEG = -1.0e30
F32 = np.float32


# ----------------------------------------------------------------- routing --
def _routing(coords):
    """Sorted order + per-query window starts (exact top-k set in 1-D)."""
    order = np.argsort(coords, kind="stable")
    cs32 = coords[order]  # compare in f32 exactly as the reference does
    w = np.zeros(S, np.int64)
    l = 0
    for p in range(S):
        lo = max(0, p - KNN + 1)
        hi = min(p, S - KNN)
        l = min(max(l, lo), hi)
        while l < hi and (cs32[p] - cs32[l]) > (cs32[l + KNN] - cs32[p]):
            l += 1
        w[p] = l
    return order, w


def _plan(w):
    """Per-quarter key-halo base, uniform slab width R and halo size KH.

    Core quarter r covers blocks 4r..4r+3; its halo starts at base[r] and
    local block j slices local keys [j*128, j*128+R).  The halo may overrun
    [0, S) -- the host zero-pads those rows (never selected by any mask).
    """
    kb = np.zeros(NBLK, np.int64)
    ends = np.zeros(NBLK, np.int64)
    for b in range(NBLK):
        ws = w[b * QB:(b + 1) * QB]
        kb[b] = (ws.min() // 128) * 128
        ends[b] = -((-(ws.max() + KNN)) // 128) * 128
    base = np.zeros(4, np.int64)
    for r in range(4):
        # may be negative (host zero-pads the halo outside [0, S))
        base[r] = min(kb[4 * r + j] - j * 128 for j in range(BPC))
    R = 0
    for r in range(4):
        for j in range(BPC):
            R = max(R, int(ends[4 * r + j] - base[r] - j * 128))
    R = max(256, -(-R // 128) * 128)
    KH = (BPC - 1) * 128 + R
    return base, R, KH


def _build_maskT(w, base, R):
    """Transposed additive masks [4, 2, R+128, 256] (bf16).

    Group g pairs blocks (2g, 2g+1); union key slab = local
    [2g*128, 2g*128 + 128 + R); columns are the 256 paired queries.
    """
    UW = R + 128
    m = np.full((4, 2, UW, 256), NEG, dtype=np.float32)
    kk = np.arange(UW)
    for r in range(4):
        for g in range(2):
            for half in range(2):
                j = 2 * g + half
                gw = w[r * QPC + j * QB: r * QPC + (j + 1) * QB]  # [128]
                # local union row of global key gk: gk - (base+2g*128)
                rel = (gw - (base[r] + 2 * g * 128))
                inside = (kk[:, None] >= rel[None, :]) &                          (kk[:, None] < rel[None, :] + KNN)
                blockcols = m[r, g, :, half * 128:(half + 1) * 128]
                blockcols[inside] = 0.0
    import ml_dtypes
    return m.astype(ml_dtypes.bfloat16)


# ------------------------------------------------------------ bass program --
def _build_nc(R, KH):
    import concourse.bass as bass
    import concourse.mybir as mybir
    from concourse import bacc
    from concourse.tile import TileContext
    from concourse.masks import make_identity

    f32 = mybir.dt.float32
    f32r = mybir.dt.float32r
    bf16 = bf16_ = mybir.dt.bfloat16
    RC = R // 128            # key chunks per slab
    KT = KH // 128           # key-halo token tiles
    UC = RC + 1              # union key chunks per block-pair group

    nc = bacc.Bacc(num_devices=8)
    xq = nc.declare_dram_parameter("xq", [QPC, D], f32, isOutput=False)
    xk = nc.declare_dram_parameter("xk", [KH, D], f32, isOutput=False)
    wqkv = nc.declare_dram_parameter("wqkv", [D, 3 * D], f32r, isOutput=False)
    bqkv = nc.declare_dram_parameter("bqkv", [3 * D], f32, isOutput=False)
    wout = nc.declare_dram_parameter("wout", [D, D], f32r, isOutput=False)
    bout = nc.declare_dram_parameter("bout", [D], f32, isOutput=False)
    maskp = nc.declare_dram_parameter("mask", [2, R + 128, 256], bf16_,
                                      isOutput=False)
    y = nc.declare_dram_parameter("y", [BPC, 128, D], f32, isOutput=True)

    with TileContext(nc) as tc:
        with (
            tc.tile_pool(name="consts", bufs=1) as consts,
            tc.tile_pool(name="big", bufs=1) as big,
            tc.tile_pool(name="xload", bufs=3) as xload,
            tc.tile_pool(name="work", bufs=3) as work,
            tc.tile_pool(name="probs", bufs=3) as probsp,
            tc.tile_pool(name="small", bufs=6) as small,
            tc.tile_pool(name="yout", bufs=3) as yout,
            tc.tile_pool(name="ps512", bufs=2, space="PSUM") as ps512,
            tc.tile_pool(name="pstr", bufs=2, space="PSUM") as pstr,
            tc.tile_pool(name="pssc", bufs=2, space="PSUM") as pssc,
            tc.tile_pool(name="psav", bufs=2, space="PSUM") as psav,
        ):
            ident = consts.tile([128, 128], f32)
            make_identity(nc, ident)
            ident_bf = consts.tile([128, 128], bf16)
            make_identity(nc, ident_bf)
            ones_f = consts.tile([1, 128], f32)
            nc.vector.memset(ones_f, 1.0)
            ones_row = consts.tile([1, 128], f32r)
            nc.vector.tensor_copy(out=ones_row, in_=ones_f)

            mask_sb = big.tile([128, 2 * UC, 256], bf16)
            wqkv_sb = consts.tile([128, 4, 3 * D], f32r)
            wout_sb = consts.tile([128, 4, D], f32r)
            bq_sb = consts.tile([128, 8], f32)
            bv_bc = consts.tile([128, D], f32)
            bo_row = consts.tile([1, D], f32r)

            xqT = big.tile([128, 4, QPC], f32r)     # queries^T
            xkT = big.tile([128, 4, KH], f32r)      # key-halo^T
            qT = big.tile([128, 4, QPC], f32r)      # head h: [(h%2)*64, h//2]
            kT = big.tile([128, 4, KH], f32r)
            # v natural + per-head ones columns: per tile [v_h(64) | 1] x 8
            vn = big.tile([128, KT, 8 * 65], bf16)
            outT = big.tile([128, 4, QPC], f32r)    # attn out^T

            # ---- phase 1: key halo first (vn gates attention) ----
            def load_transpose(src, n_tiles, dst):
                for tt in range(n_tiles):
                    xt = xload.tile([128, D], f32, tag="xt")
                    nc.sync.dma_start(out=xt,
                                      in_=src[tt * 128:(tt + 1) * 128, :])
                    tp = pstr.tile([128, 4, 128], f32, tag="tr")
                    for kc in range(4):
                        nc.tensor.transpose(
                            tp[:, kc, :], xt[:, kc * 128:(kc + 1) * 128],
                            ident)
                    if tt % 2 == 0:
                        nc.vector.tensor_copy(
                            out=dst[:, :, tt * 128:(tt + 1) * 128],
                            in_=tp[:, :, :])
                    else:
                        nc.scalar.copy(
                            out=dst[:, :, tt * 128:(tt + 1) * 128],
                            in_=tp[:, :, :])

            load_transpose(xk, KT, xkT)
            # v weights + bias, then v projection (dense, warms the PE)
            nc.sync.dma_start(
                out=wqkv_sb[:, :, 2 * D:3 * D],
                in_=wqkv.rearrange("(kc p) n -> p kc n", p=128)[:, :,
                                                              2 * D:3 * D])
            nc.sync.dma_start(
                out=bv_bc,
                in_=bass.AP(tensor=bqkv, offset=1024, ap=[[0, 128], [1, D]]))
            nc.sync.dma_start(
                out=bq_sb,
                in_=bass.AP(tensor=bqkv, offset=0, ap=[[1, 128], [128, 8]]))
            for tt in range(KT):
                ps = ps512.tile([128, 512], f32, tag="mm512")
                for kc in range(4):
                    nc.tensor.matmul(
                        ps, lhsT=xkT[:, kc, tt * 128:(tt + 1) * 128],
                        rhs=wqkv_sb[:, kc, 2 * D:3 * D],
                        start=(kc == 0), stop=(kc == 3))
                dst = vn[:, tt, :]
                seg = bass.AP(tensor=dst.tensor, offset=dst.offset,
                              ap=[dst.ap[0], [65, 8], [1, 64]])
                nc.vector.tensor_tensor(
                    out=seg,
                    in0=ps.rearrange("p (h c) -> p h c", h=8),
                    in1=bv_bc.rearrange("p (h c) -> p h c", h=8),
                    op=mybir.AluOpType.add)
            ones_ap = bass.AP(tensor=vn.tensor, offset=vn.offset + 64,
                              ap=[vn.ap[0], [8 * 65, KT], [65, 8]])
            nc.vector.memset(ones_ap, 1.0)

            load_transpose(xq, QPC // 128, xqT)
            nc.sync.dma_start(
                out=mask_sb,
                in_=maskp.rearrange("g (kk p) q -> p (g kk) q", p=128))
            # out-proj weights + bias (needed last)
            nc.sync.dma_start(
                out=wout_sb, in_=wout.rearrange("(kc p) n -> p kc n", p=128))
            bo_f = consts.tile([1, D], f32)
            nc.sync.dma_start(
                out=bo_f,
                in_=bass.AP(tensor=bout, offset=0, ap=[[0, 1], [1, D]]))
            nc.vector.tensor_copy(out=bo_row, in_=bo_f)

            # ---- q/k projection for one head-pair (feature chunk m) ----
            def project_qk(m):
                nc.sync.dma_start(
                    out=wqkv_sb[:, :, m * 128:(m + 1) * 128],
                    in_=wqkv.rearrange("(kc p) n -> p kc n", p=128)
                    [:, :, m * 128:(m + 1) * 128])
                nc.sync.dma_start(
                    out=wqkv_sb[:, :, D + m * 128: D + (m + 1) * 128],
                    in_=wqkv.rearrange("(kc p) n -> p kc n", p=128)
                    [:, :, D + m * 128: D + (m + 1) * 128])
                for t, (src, src_w, dst) in enumerate(
                        ((xqT, QPC, qT), (xkT, KH, kT))):
                    nf = (src_w + 511) // 512
                    for f in range(nf):
                        fw = min(512, src_w - f * 512)
                        ps = ps512.tile([128, 512], f32, tag="mm512")
                        for kc in range(4):
                            nc.tensor.matmul(
                                ps[:, :fw],
                                lhsT=wqkv_sb[:, kc,
                                             t * D + m * 128:
                                             t * D + (m + 1) * 128],
                                rhs=src[:, kc, f * 512: f * 512 + fw],
                                start=(kc == 0), stop=(kc == 3))
                        nc.vector.tensor_scalar_add(
                            out=dst[:, m, f * 512: f * 512 + fw],
                            in0=ps[:, :fw],
                            scalar1=bq_sb[:, 4 * t + m: 4 * t + m + 1])

            # ---- attention stages ----
            st = {}
            ot_tiles = {}

            def stage_G(t, bg, h):
                po, ch = (h % 2) * 64, h // 2
                ptsb = probsp.tile([128, UC, 256], bf16, tag="ptsb")
                for kp in range((UC + 1) // 2):
                    w2 = min(2, UC - kp * 2)
                    sc = pssc.tile([128, 2, 256], f32, tag="sc")
                    for i in range(w2):
                        kk = kp * 2 + i
                        nc.tensor.matmul(
                            sc[:, i, :],
                            lhsT=kT[po:po + 64, ch,
                                    (2 * bg + kk) * 128:
                                    (2 * bg + kk + 1) * 128],
                            rhs=qT[po:po + 64, ch, bg * 256:(bg + 1) * 256],
                            start=True, stop=False)
                        nc.tensor.matmul(sc[:, i, :], lhsT=ident_bf,
                                         rhs=mask_sb[:, bg * UC + kk, :],
                                         start=False, stop=True)
                    nc.scalar.activation(
                        out=ptsb[:, kp * 2: kp * 2 + w2, :],
                        in_=sc[:, :w2, :],
                        func=mybir.ActivationFunctionType.Exp,
                        scale=float(SCALE))
                st[t] = ptsb

            def stage_A(t, bg, h, half):
                j = 2 * bg + half
                ptsb = st[t]
                av = psav.tile([128, 65], f32, tag="av")
                for ck in range(RC):
                    nc.tensor.matmul(
                        av,
                        lhsT=ptsb[:, half + ck, half * 128:(half + 1) * 128],
                        rhs=vn[:, j + ck, h * 65: h * 65 + 65],
                        start=(ck == 0), stop=(ck == RC - 1))
                recip = small.tile([128, 1], f32, tag="recip")
                nc.vector.reciprocal(out=recip, in_=av[:, 64:65])
                outq = work.tile([128, 64], f32, tag="outq")
                if (t + half) % 2 == 0:
                    nc.vector.tensor_scalar_mul(out=outq, in0=av[:, 0:64],
                                                scalar1=recip)
                else:
                    nc.scalar.mul(out=outq, in_=av[:, 0:64], mul=recip)
                if h % 2 == 0:
                    ot = pstr.tile([64, 256], f32, tag="tr")
                    ot_tiles[half] = ot
                ot = ot_tiles[half]
                nc.tensor.transpose(ot[:, (h % 2) * 128:(h % 2 + 1) * 128],
                                    outq, ident)
                if h % 2 == 1:
                    ch = h // 2
                    if (t + half) % 2 == 0:
                        nc.vector.tensor_copy(
                            out=outT[0:64, ch, j * 128:(j + 1) * 128],
                            in_=ot[:, 0:128])
                        nc.scalar.copy(
                            out=outT[64:128, ch, j * 128:(j + 1) * 128],
                            in_=ot[:, 128:256])
                    else:
                        nc.scalar.copy(
                            out=outT[0:64, ch, j * 128:(j + 1) * 128],
                            in_=ot[:, 0:128])
                        nc.vector.tensor_copy(
                            out=outT[64:128, ch, j * 128:(j + 1) * 128],
                            in_=ot[:, 128:256])
                    del ot_tiles[half]
                # after the last head of block j: local out-proj
                if h == HEADS - 1:
                    ps = ps512.tile([128, 512], f32, tag="mm512")
                    for kc in range(4):
                        nc.tensor.matmul(
                            ps, lhsT=outT[:, kc, j * 128:(j + 1) * 128],
                            rhs=wout_sb[:, kc, :],
                            start=(kc == 0), stop=False)
                    nc.tensor.matmul(ps, lhsT=ones_row, rhs=bo_row,
                                     start=False, stop=True)
                    ysb = yout.tile([128, D], f32, tag="ysb")
                    if j % 2 == 0:
                        nc.vector.tensor_copy(out=ysb, in_=ps)
                    else:
                        nc.scalar.copy(out=ysb, in_=ps)
                    nc.sync.dma_start(out=y[j], in_=ysb)

            # hp-major order interleaves the dense q/k projections with
            # attention groups (keeps the PE activity monitor warm)
            glist = []
            for hp in range(4):
                for bg in range(2):
                    for h in (2 * hp, 2 * hp + 1):
                        glist.append((bg, h))
            NG = len(glist)
            for t in range(NG + 1):
                if t < NG:
                    if t % 4 == 0:
                        project_qk(t // 4)
                    stage_G(t, *glist[t])
                if 0 <= t - 1 < NG:
                    bg, h = glist[t - 1]
                    stage_A(t - 1, bg, h, 0)
                    stage_A(t - 1, bg, h, 1)
                    del st[t - 1]

    nc.finalize()
    return nc


_CACHE = {}
_LAST_NC = None
_LAST_IN_MAPS = None


def _get_nc(R, KH):
    key = (R, KH)
    if key not in _CACHE:
        _CACHE[key] = _build_nc(R, KH)
    return _CACHE[key]


# ---------------------------------------------------------------- kernel ----
def kernel(x, cantor_coords, W_qkv, b_qkv, W_out, b_out):
    from concourse.bass_utils import run_bass_kernel_spmd

    x = np.ascontiguousarray(x, dtype=F32)
    coords = np.ascontiguousarray(cantor_coords, dtype=F32)
    W_qkv = np.ascontiguousarray(W_qkv, dtype=F32)
    b_qkv = np.ascontiguousarray(b_qkv, dtype=F32)
    W_out = np.ascontiguousarray(W_out, dtype=F32)
    b_out = np.ascontiguousarray(b_out, dtype=F32)
    B = x.shape[0]
    assert x.shape == (B, S, D) and coords.shape == (S,)

    order, w = _routing(coords)
    base, R, KH = _plan(w)
    assert R <= 512 and R % 128 == 0, R
    assert KH % 128 == 0 and KH <= 1536, KH
    masks = _build_maskT(w, base, R)
    nc = _get_nc(R, KH)

    xs = {b: x[b][order] for b in range(B)}
    in_maps = []
    for c in range(8):
        b, r = c // 4, c % 4
        lo = int(base[r])
        halo = np.zeros((KH, D), dtype=F32)
        s0, s1 = max(0, lo), min(S, lo + KH)
        halo[s0 - lo: s1 - lo] = xs[b][s0:s1]
        in_maps.append({
            "xq": np.ascontiguousarray(xs[b][r * QPC:(r + 1) * QPC]),
            "xk": halo,
            "wqkv": W_qkv,
            "bqkv": b_qkv,
            "wout": W_out,
            "bout": b_out,
            "mask": np.ascontiguousarray(masks[r]),  # [2, R+128, 256] bf16
        })

    nc_obj = nc
    global _LAST_NC, _LAST_IN_MAPS
    _LAST_NC, _LAST_IN_MAPS = nc_obj, in_maps
    res = run_bass_kernel_spmd(nc_obj, in_maps, list(range(8))).results

    out = np.empty((B, S, D), dtype=F32)
    ys = np.empty((S, D), dtype=F32)
    for b in range(B):
        for r in range(4):
            yr = res[4 * b + r]["y"]          # [4, 128, D]
            ys[r * QPC:(r + 1) * QPC] = yr.reshape(QPC, D)
        out[b][order] = ys
    return out
